# revision 11
# baseline (speedup 1.0000x reference)
"""Trainium2 Bass kernel for MultiHeadSelfAttention (cross-attention variant).

Problem: B=2, LQ=LK=2048, D=1024, H=16, d_k=64, fp32.
  q_a = cdd @ W_q + b_q ; k_a = his @ W_k + b_k ; v_a = his @ W_v + b_v
  S = q k^T / 8 ; A = exp(S) / (sum_k exp(S) + 1e-8) ; ctx = A v
  returns (context, q_a)

Sharding (8 cores, no collectives): core c handles batch c//4 and head-block
c%4 (4 heads = 256 columns of W_q/W_k/W_v).  Each core writes disjoint column
slices of both outputs; the host gathers them.

The kernel is ACT(exp)-bound: 16.8M exps/core at 1 elem/cycle/lane @1.2GHz.
Design goals: (1) few, large exp instructions (amortize the ~220cy fixed
cost per ACTIVATE), (2) ACT never idles — attention starts as soon as the
first kv chunks are projected (the custom softmax has no max subtraction, so
ctx/denominator accumulate over k-chunks in SBUF, flash-attention style).

Structure (k-outer): 32 units = (k half: 8 k-tiles) x (4 q-chunks) x (4
heads).  Per unit: score groups (3,3,2) k-tiles -> exp FD (1536,1536,1024)
from PSUM tiles spA/spB (3 banks each); MM2 accumulates ctx^T[65,512] over
the unit's 8 k-tiles in a 1-bank cp region (the V ones-column gives row
sums); one DVE drain adds it into the SBUF accumulator.  After the second
sweep, normalize: PE-transpose [65,128] blocks into the util bank, DVE
reciprocal, scale into a token-major staging tile, DMA out.
kv chunks 0-1 + q chunk 0 load eagerly (transpose drains on the then-idle
ACT); kv 2-3 / q 1-3 drip into attention's PE+DVE slack.
"""

import numpy as np
from contextlib import ExitStack

B = 2
L = 2048
D = 1024
H = 16
DK = 64
P = 128
NCORES = 8
CPB = 4  # cores per batch
HPC = H // CPB  # heads per core = 4
COLS = HPC * DK  # 256 output columns per core
CHUNK = 512  # token chunk (max fp32 moving operand)

_CACHE = {}


def _build(L=L, D=D, COLS=COLS, cfg=None, repeat=1):
    import concourse.tile as tile
    from concourse import bacc, masks, mybir

    f32 = mybir.dt.float32
    f32r = mybir.dt.float32r
    Exp = mybir.ActivationFunctionType.Exp
    add_op = mybir.AluOpType.add
    mult_op = mybir.AluOpType.mult

    HL = COLS // DK  # heads handled locally = 4
    FT = D // P  # feature tiles = 8
    TT = L // P  # k token tiles = 16
    TCH = L // CHUNK  # token chunks = 4
    CT = COLS // P  # column tiles = 2
    IT = CHUNK // P  # token tiles per chunk = 4
    VW = DK + 1  # 65: V columns + ones column
    KH = TT // 2  # k-tiles per half (unit) = 8
    GRP = (3, 3, 2)  # k-tiles per exp call within a unit

    cfg = dict(
        dict(
            order="kouter",  # kouter | fine
            eager_kv=2,      # kv chunks loaded before attention
            drip_sp=True,    # drip may borrow spA/spB bank 2
            norm_eng="vector",  # pool | vector for the normalize multiply
            qa_psum_dma=False,  # DMA q_a straight from PSUM
            dvet_kv=(),  # kv chunks loaded via DVE stream-transpose
            dvet_q=(),   # q chunks loaded via DVE stream-transpose
            lag=1,
        ),
        **(cfg or {}),
    )

    nc = bacc.Bacc(
        "TRN2",
        target_bir_lowering=False,
        debug=False,
        num_devices=NCORES,
    )

    x_q = nc.dram_tensor("x_q", [L, D], f32, kind="ExternalInput").ap()
    x_kv = nc.dram_tensor("x_kv", [L, D], f32, kind="ExternalInput").ap()
    w_q = nc.dram_tensor("w_q", [D, COLS], f32, kind="ExternalInput").ap()
    w_k = nc.dram_tensor("w_k", [D, COLS], f32, kind="ExternalInput").ap()
    w_v = nc.dram_tensor("w_v", [D, COLS], f32, kind="ExternalInput").ap()
    b_q = nc.dram_tensor("b_q", [COLS], f32, kind="ExternalInput").ap()
    b_k = nc.dram_tensor("b_k", [COLS], f32, kind="ExternalInput").ap()
    b_v = nc.dram_tensor("b_v", [COLS], f32, kind="ExternalInput").ap()
    q_out = nc.dram_tensor("q_out", [L, COLS], f32, kind="ExternalOutput").ap()
    c_out = nc.dram_tensor("c_out", [L, COLS], f32, kind="ExternalOutput").ap()

    with tile.TileContext(nc) as tc, ExitStack() as ctx:
        singles = ctx.enter_context(tc.tile_pool(name="singles", bufs=1))

        identity = singles.tile([P, P], f32)
        masks.make_identity(nc, identity[:])

        # biases: q/k as per-partition scalars in ^T layout; v broadcast to rows
        bq_sb = singles.tile([P, CT], f32)
        bk_sb = singles.tile([P, CT], f32)
        nc.sync.dma_start(bq_sb[:], b_q.rearrange("(c p) -> p c", p=P))
        nc.sync.dma_start(bk_sb[:], b_k.rearrange("(c p) -> p c", p=P))
        bv_row = singles.tile([1, COLS], f32)
        nc.sync.dma_start(bv_row[:], b_v.rearrange("(o c) -> o c", o=1))
        bv_bcast = singles.tile([P, COLS], f32)
        nc.gpsimd.partition_broadcast(bv_bcast[:], bv_row[:1])

        # weights: [D, COLS] -> [128, FT, COLS], rounded to f32r via DVE
        wq_sb = singles.tile([P, FT * COLS], f32r)
        wk_sb = singles.tile([P, FT * COLS], f32r)
        wv_sb = singles.tile([P, FT * COLS], f32r)
        wq_sb = wq_sb.rearrange("p (f c) -> p f c", f=FT)
        wk_sb = wk_sb.rearrange("p (f c) -> p f c", f=FT)
        wv_sb = wv_sb.rearrange("p (f c) -> p f c", f=FT)
        wstage_pool = ctx.enter_context(tc.tile_pool(name="wstage", bufs=1))
        for wsb, wdr in ((wq_sb, w_q), (wk_sb, w_k), (wv_sb, w_v)):
            wst = wstage_pool.tile([P, FT * COLS], f32, tag="wst")
            wst = wst.rearrange("p (f c) -> p f c", f=FT)
            nc.sync.dma_start(wst[:], wdr.rearrange("(f p) c -> p f c", p=P))
            nc.vector.tensor_copy(wsb[:], wst[:])

        # persistent attention operands
        QT = singles.tile([P, CT * L], f32r)
        KT = singles.tile([P, CT * L], f32r)
        V = singles.tile([P, TT * HL * VW], f32r)
        QT = QT.rearrange("p (c l) -> p c l", c=CT)
        KT = KT.rearrange("p (c l) -> p c l", c=CT)
        V = V.rearrange("p (t h w) -> p t h w", t=TT, h=HL)
        ones1 = singles.tile([P, 1], f32)
        nc.vector.memset(ones1[:], 1.0)
        nc.vector.tensor_copy(
            V[:, :, :, DK : DK + 1], ones1[:].to_broadcast((P, TT, HL, 1))
        )

        # ctx^T accumulator in SBUF: [65, (qc, h, 512)]
        ACC = singles.tile([VW, TCH * HL * CHUNK], f32)
        ACC = ACC.rearrange("p (q h l) -> p q h l", q=TCH, h=HL)

        # ---- PSUM: spA(3) + spB(3) + cp(1) + util(1) = 8 banks ----
        psum = ctx.enter_context(tc.tile_pool(name="psum", bufs=1, space="PSUM"))
        SPN = 3 * CHUNK  # 1536
        spA = psum.tile([P, SPN], f32, tag="spA")
        spB = psum.tile([P, SPN], f32, tag="spB")
        cp = psum.tile([P, CHUNK], f32, tag="cp")
        util = psum.tile([P, CHUNK], f32, tag="util")
        sp = [spA, spB]

        espool = ctx.enter_context(tc.tile_pool(name="es", bufs=2))
        xnat_pool = ctx.enter_context(tc.tile_pool(name="xnat", bufs=4))
        xt_pool = ctx.enter_context(tc.tile_pool(name="xt", bufs=2))
        qnat_pool = ctx.enter_context(tc.tile_pool(name="qnat", bufs=3))
        nrm_pool = ctx.enter_context(tc.tile_pool(name="nrm", bufs=2))
        rec_pool = ctx.enter_context(tc.tile_pool(name="rec", bufs=2))
        ctxb_pool = ctx.enter_context(tc.tile_pool(name="ctxb", bufs=2))

        # ---- PSUM slice provider --------------------------------------
        # Fill phase rotates over all 8 banks; during attention the drip
        # rotates util (+ the spare third banks of spA/spB, which the (3,3,2)
        # group pattern leaves idle most of the time).
        class Provider:
            def __init__(self):
                self.fill = [
                    spA[:, 0:CHUNK], spB[:, 0:CHUNK],
                    spA[:, CHUNK : 2 * CHUNK], spB[:, CHUNK : 2 * CHUNK],
                    spA[:, 2 * CHUNK : 3 * CHUNK], spB[:, 2 * CHUNK : 3 * CHUNK],
                    cp[:, 0:CHUNK], util[:, 0:CHUNK],
                ]
                if cfg["drip_sp"]:
                    self.drip = [
                        util[:, 0:CHUNK],
                        spA[:, 2 * CHUNK : 3 * CHUNK],
                        spB[:, 2 * CHUNK : 3 * CHUNK],
                    ]
                else:
                    self.drip = [util[:, 0:CHUNK]]
                self.lst = self.fill
                self.i = 0

            def mode(self, name):
                self.lst = getattr(self, name)
                self.i = 0

            def get(self):
                s = self.lst[self.i % len(self.lst)]
                self.i += 1
                return s

        prov = Provider()

        # ---- building blocks -------------------------------------------
        def lt_load(xdram, tag, ch):
            tok0 = ch * CHUNK
            xnats = []
            for it in range(IT):
                xn = xnat_pool.tile([P, D], f32, tag="xn", name=f"xn{tag}")
                nc.sync.dma_start(
                    xn[:], xdram[tok0 + it * P : tok0 + (it + 1) * P, :]
                )
                xnats.append(xn)
            xt = xt_pool.tile([P, FT * CHUNK], f32r, tag="xt", name=f"xt{tag}")
            xt = xt.rearrange("p (f l) -> p f l", f=FT)
            return xnats, xt

        def lt_transpose(xnats, xt, ft, drain):
            tp = prov.get()
            for it in range(IT):
                nc.tensor.transpose(
                    tp[:, it * P : (it + 1) * P],
                    xnats[it][:, ft * P : (ft + 1) * P],
                    identity[:],
                )
            drain(xt[:, ft, :], tp[:])

        def load_transpose(xdram, tag, ch, drain):
            xnats, xt = lt_load(xdram, tag, ch)
            for ft in range(FT):
                lt_transpose(xnats, xt, ft, drain)
            return xt

        def load_transpose_dve(xdram, tag, ch):
            """Block-swapped DMA load + DVE StreamTranspose (no PE/PSUM/ACT).

            dst S[32A+v, 32B+u] = X[tok0+32B+v, 128ft+32A+u]; stream-transpose
            of 32x32 blocks then yields X^T exactly.
            """
            tok0 = ch * CHUNK
            xt = xt_pool.tile([P, FT * CHUNK], f32r, tag="xt", name=f"xt{tag}")
            xt = xt.rearrange("p (f l) -> p f l", f=FT)
            for ft in range(FT):
                s = xnat_pool.tile([P, CHUNK], f32, tag="xs", name=f"xs{tag}")
                blk = xdram[tok0 : tok0 + CHUNK, ft * P : (ft + 1) * P]
                swz = blk.rearrange("(b v) (a u) -> a v b u", v=32, u=32)
                dst = s.rearrange("p (b u) -> p b u", u=32)
                for a in range(4):
                    nc.sync.dma_start(dst[a * 32 : (a + 1) * 32], swz[a])
                nc.vector.transpose(xt[:, ft, :], s[:])
            return xt

        def proj_T_ct(wsb, xt, bsb, OUT, ch, ct):
            tok0 = ch * CHUNK
            pp = prov.get()
            for ft in range(FT):
                nc.tensor.matmul(
                    pp[:],
                    wsb[:, ft, ct * P : (ct + 1) * P],
                    xt[:, ft, :],
                    start=(ft == 0),
                    stop=(ft == FT - 1),
                )
            nc.vector.tensor_scalar_add(
                OUT[:, ct, tok0 : tok0 + CHUNK], pp[:], bsb[:, ct : ct + 1]
            )

        def proj_v_it(xt_kv, ch, it):
            pv = prov.get()
            for ft in range(FT):
                nc.tensor.matmul(
                    pv[:, :COLS],
                    xt_kv[:, ft, it * P : (it + 1) * P],
                    wv_sb[:, ft, :],
                    start=(ft == 0),
                    stop=(ft == FT - 1),
                )
            nc.vector.tensor_tensor(
                V[:, ch * IT + it, :, 0:DK],
                pv[:, :COLS].rearrange("p (h w) -> p h w", h=HL),
                bv_bcast[:].rearrange("p (h w) -> p h w", h=HL),
                op=add_op,
            )

        def qa_out_one(ch, ct, it):
            tok0 = ch * CHUNK
            tq = prov.get()
            nc.tensor.transpose(
                tq[:, :P],
                QT[:, ct, tok0 + it * P : tok0 + (it + 1) * P].bitcast(f32),
                identity[:],
            )
            if cfg["qa_psum_dma"]:
                nc.sync.dma_start(
                    q_out[tok0 + it * P : tok0 + (it + 1) * P, ct * P : (ct + 1) * P],
                    tq[:, :P],
                )
            else:
                qn = qnat_pool.tile([P, P], f32, tag="qn", name="qn")
                nc.vector.tensor_copy(qn[:], tq[:, :P])
                nc.sync.dma_start(
                    q_out[tok0 + it * P : tok0 + (it + 1) * P, ct * P : (ct + 1) * P],
                    qn[:],
                )

        # ---- attention unit: (khalf, qc, h), 8 k-tiles, groups (3,3,2) ---
        tilesel = [0]  # alternates spA/spB per score group, globally

        def unit(kh, qc, h, feed=None):
            q0 = qc * CHUNK
            ct, hh = divmod(h, HL // CT)
            rows = slice(hh * DK, (hh + 1) * DK)
            kt0 = kh * KH

            def mm2(kts, es, es_off):
                for j, kt in enumerate(kts):
                    nc.tensor.matmul(
                        cp[:VW, :],
                        V[:, kt, h, :],
                        es[:, es_off + j * CHUNK : es_off + (j + 1) * CHUNK],
                        start=(kt == kt0),
                        stop=(kt == kt0 + KH - 1),
                    )

            pend = []
            off = 0
            for g in GRP:
                kts = list(range(kt0 + off, kt0 + off + g))
                t = sp[tilesel[0]]
                tilesel[0] ^= 1
                for j, kt in enumerate(kts):
                    nc.tensor.matmul(
                        t[:, j * CHUNK : (j + 1) * CHUNK],
                        KT[rows, ct, kt * P : (kt + 1) * P],
                        QT[rows, ct, q0 : q0 + CHUNK],
                        start=True,
                        stop=True,
                    )
                es = espool.tile([P, SPN], f32r, tag="es", name="es")
                nc.scalar.activation(
                    es[:, : g * CHUNK], t[:, : g * CHUNK], Exp, scale=0.125
                )
                pend.append((kts, es, 0))
                if len(pend) > cfg["lag"]:
                    mm2(*pend.pop(0))
                    if feed is not None:
                        feed()
                if feed is not None:
                    feed()
                off += g
            while pend:
                mm2(*pend.pop(0))
                if feed is not None:
                    feed()
            # drain ctx^T partial into the SBUF accumulator
            if kh == 0:
                nc.vector.tensor_copy(ACC[:, qc, h, :], cp[:VW, :])
            else:
                nc.vector.tensor_tensor(
                    ACC[:, qc, h, :], cp[:VW, :], ACC[:, qc, h, :], op=add_op
                )

        def normalize(qc, h, ctxbuf):
            """ACC[:, qc, h] -> token-major normalized ctx in ctxbuf."""
            tn = prov.get()
            for it in range(IT):
                nc.tensor.transpose(
                    tn[:, it * VW : (it + 1) * VW],
                    ACC[:VW, qc, h, it * P : (it + 1) * P],
                    identity[:VW, :VW],
                )
            nrm = nrm_pool.tile([P, IT * VW], f32, tag="nrm", name="nrm")
            nc.vector.tensor_copy(nrm[:], tn[:, : IT * VW])
            nrmv = nrm[:].rearrange("p (i w) -> p i w", i=IT)
            rec = rec_pool.tile([P, 2 * IT], f32, tag="rec", name="rec")
            recv = rec[:].rearrange("p (x i) -> p x i", x=2)
            nc.vector.tensor_scalar_add(recv[:, 0, :], nrmv[:, :, DK], 1e-8)
            nc.vector.reciprocal(recv[:, 1, :], recv[:, 0, :])
            eng = nc.gpsimd if cfg["norm_eng"] == "pool" else nc.vector
            eng.tensor_tensor(
                ctxbuf[:].rearrange("p (i c) -> p i c", i=IT)[
                    :, :, h * DK : (h + 1) * DK
                ],
                nrmv[:, :, 0:DK],
                recv[:, 1:2, :]
                .rearrange("p x i -> p i x")
                .to_broadcast((P, IT, DK)),
                op=mult_op,
            )

        # ---- drip-unit machinery ----------------------------------------
        # units_q: (gate, closure) FIFO of projection work dripped into
        # attention slack.  units_norm: priority queue of deferred
        # normalize/output work (consumed first — cheap, unblocks SBUF).
        units_q = []
        units_norm = []

        def dvet_ft(xdram, xt, ch, ft, tag):
            tok0 = ch * CHUNK
            s = xnat_pool.tile([P, CHUNK], f32, tag="xs", name=f"xs{tag}")
            blk = xdram[tok0 : tok0 + CHUNK, ft * P : (ft + 1) * P]
            swz = blk.rearrange("(b v) (a u) -> a v b u", v=32, u=32)
            dst = s.rearrange("p (b u) -> p b u", u=32)
            for a in range(4):
                nc.sync.dma_start(dst[a * 32 : (a + 1) * 32], swz[a])
            nc.vector.transpose(xt[:, ft, :], s[:])

        def q_side_units(ch):
            g = f"q{ch}"
            if ch in cfg["dvet_q"]:
                xt = xt_pool.tile([P, FT * CHUNK], f32r, tag="xt", name="xtq")
                xt = xt.rearrange("p (f l) -> p f l", f=FT)
                for ft in range(FT):
                    units_q.append(
                        (g, lambda x=xt, f=ft, k=ch: dvet_ft(x_q, x, k, f, "q"))
                    )
            else:
                xnats, xt = lt_load(x_q, "q", ch)
                for ft in range(FT):
                    units_q.append(
                        (g, lambda xn=xnats, x=xt, f=ft: lt_transpose(
                            xn, x, f, nc.vector.tensor_copy
                        ))
                    )
            for ct in range(CT):
                units_q.append(
                    (g, lambda x=xt, c=ct, k=ch: proj_T_ct(wq_sb, x, bq_sb, QT, k, c))
                )
            for ct in range(CT):
                for it in range(IT):
                    units_q.append(
                        (g, lambda k=ch, c=ct, i=it: qa_out_one(k, c, i))
                    )

        def kv_side_units(ch):
            g = f"kv{ch}"
            if ch in cfg["dvet_kv"]:
                xt = xt_pool.tile([P, FT * CHUNK], f32r, tag="xt", name="xtkv")
                xt = xt.rearrange("p (f l) -> p f l", f=FT)
                for ft in range(FT):
                    units_q.append(
                        (g, lambda x=xt, f=ft, k=ch: dvet_ft(x_kv, x, k, f, "kv"))
                    )
            else:
                xnats, xt = lt_load(x_kv, "kv", ch)
                for ft in range(FT):
                    units_q.append(
                        (g, lambda xn=xnats, x=xt, f=ft: lt_transpose(
                            xn, x, f, nc.vector.tensor_copy
                        ))
                    )
            for ct in range(CT):
                units_q.append(
                    (g, lambda x=xt, c=ct, k=ch: proj_T_ct(wk_sb, x, bk_sb, KT, k, c))
                )
            for it in range(IT):
                units_q.append((g, lambda x=xt, k=ch, i=it: proj_v_it(x, k, i)))

        def feed():
            if units_norm:
                units_norm.pop(0)()
            elif units_q:
                units_q.pop(0)[1]()

        def flush(gate):
            """Emit all queued drip work up to and including `gate` —
            required before any unit that reads what the gate produces."""
            while any(g == gate for g, _ in units_q):
                units_q.pop(0)[1]()

        # ---- emission ----------------------------------------------------
        def kv_eager(ch):
            if ch in cfg["dvet_kv"]:
                xt_kv = load_transpose_dve(x_kv, "kv", ch)
            else:
                xt_kv = load_transpose(x_kv, "kv", ch, drain=nc.scalar.copy)
            for ct in range(CT):
                proj_T_ct(wk_sb, xt_kv, bk_sb, KT, ch, ct)
            for it in range(IT):
                proj_v_it(xt_kv, ch, it)

        def emit_kouter():
            EK = cfg["eager_kv"]
            for ch in range(EK):
                kv_eager(ch)
            if 0 in cfg["dvet_q"]:
                xt_q0 = load_transpose_dve(x_q, "q", 0)
            else:
                xt_q0 = load_transpose(x_q, "q", 0, drain=nc.scalar.copy)
            for ct in range(CT):
                proj_T_ct(wq_sb, xt_q0, bq_sb, QT, 0, ct)
            prov.mode("drip")
            for ct in range(CT):
                for it in range(IT):
                    qa_out_one(0, ct, it)
            # drip order matched to first use: q(qc) gates sweep-0 unit 4*qc,
            # kv(2) gates sweep-1 start, kv(3) shortly after
            q_side_units(1)
            q_side_units(2)
            if EK < 3:
                kv_side_units(2)
            q_side_units(3)
            if EK < 4:
                kv_side_units(3)
            for ch in range(4, TCH):
                kv_side_units(ch)
            # sweep 0 (k-tiles 0..7), then sweep 1 (8..15) with normalize
            for qc in range(TCH):
                if qc > 0:
                    flush(f"q{qc}")
                for h in range(HL):
                    unit(0, qc, h, feed=feed)
            for ch in range(EK, TCH):
                flush(f"kv{ch}")
            for qc in range(TCH):
                ctxbuf = ctxb_pool.tile(
                    [P, IT * COLS], f32, tag="ctxb", name="ctxb"
                )
                for h in range(HL):
                    unit(1, qc, h, feed=feed)
                    # defer normalize so it doesn't head-of-line block the
                    # next unit's score matmuls in the PE queue
                    units_norm.append(
                        lambda q=qc, hh=h, cb=ctxbuf: normalize(q, hh, cb)
                    )
                def ctx_dma(q=qc, cb=ctxbuf):
                    cbv = cb[:].rearrange("p (i c) -> p i c", i=IT)
                    for it in range(IT):
                        nc.sync.dma_start(
                            c_out[q * CHUNK + it * P : q * CHUNK + (it + 1) * P, :],
                            cbv[:, it, :],
                        )
                units_norm.append(ctx_dma)
            while units_norm:
                units_norm.pop(0)()
            while units_q:
                units_q.pop(0)[1]()

        for _rep in range(repeat):
            tilesel[0] = 0
            prov.mode("fill")
            emit_kouter()
    nc.compile()
    return nc


def _get_nc():
    if "nc" not in _CACHE:
        _CACHE["nc"] = _build(cfg=_CACHE.get("cfg"))
    return _CACHE["nc"]


def make_in_maps(cdd, his, W_q, b_q, W_k, b_k, W_v, b_v):
    cdd = np.asarray(cdd, dtype=np.float32)
    his = np.asarray(his, dtype=np.float32)
    W_q = np.asarray(W_q, dtype=np.float32)
    W_k = np.asarray(W_k, dtype=np.float32)
    W_v = np.asarray(W_v, dtype=np.float32)
    b_q = np.asarray(b_q, dtype=np.float32)
    b_k = np.asarray(b_k, dtype=np.float32)
    b_v = np.asarray(b_v, dtype=np.float32)
    in_maps = []
    for c in range(NCORES):
        b, hb = divmod(c, CPB)
        sl = slice(hb * COLS, (hb + 1) * COLS)
        in_maps.append(
            {
                "x_q": np.ascontiguousarray(cdd[b]),
                "x_kv": np.ascontiguousarray(his[b]),
                "w_q": np.ascontiguousarray(W_q[:, sl]),
                "w_k": np.ascontiguousarray(W_k[:, sl]),
                "w_v": np.ascontiguousarray(W_v[:, sl]),
                "b_q": np.ascontiguousarray(b_q[sl]),
                "b_k": np.ascontiguousarray(b_k[sl]),
                "b_v": np.ascontiguousarray(b_v[sl]),
            }
        )
    return in_maps


def assemble_outputs(results):
    context = np.zeros((B, L, D), dtype=np.float32)
    q_a = np.zeros((B, L, D), dtype=np.float32)
    for c, out in enumerate(results):
        b, hb = divmod(c, CPB)
        sl = slice(hb * COLS, (hb + 1) * COLS)
        q_a[b, :, sl] = out["q_out"]
        context[b, :, sl] = out["c_out"]
    return (context, q_a)


def kernel(cdd, his, W_q, b_q, W_k, b_k, W_v, b_v):
    from concourse.bass_utils import run_bass_kernel_spmd

    nc = _get_nc()
    in_maps = make_in_maps(cdd, his, W_q, b_q, W_k, b_k, W_v, b_v)

    res = run_bass_kernel_spmd(
        nc, in_maps, core_ids=list(range(NCORES)), trace=_CACHE.get("trace", False)
    )
    _CACHE["last_result"] = res
    return assemble_outputs(res.results)


# revision 29
# speedup vs baseline: 1.4547x; 1.4547x over previous
"""Trainium2 Bass kernel for MultiHeadSelfAttention (cross-attention variant).

Problem: B=2, LQ=LK=2048, D=1024, H=16, d_k=64, fp32.
  q_a = cdd @ W_q + b_q ; k_a = his @ W_k + b_k ; v_a = his @ W_v + b_v
  S = q k^T / 8 ; A = exp(S) / (sum_k exp(S) + 1e-8) ; ctx = A v
  returns (context, q_a)

Sharding (8 cores, no collectives): core c handles batch c//4 and head-block
c%4 (4 heads = 256 columns of W_q/W_k/W_v).  Each core writes disjoint column
slices of both outputs; the host gathers them.

The kernel is ACT(exp)-bound: 16.8M exps/core at 1 elem/cycle/lane @1.2GHz
(~109us floor; each ACTIVATE costs ~(FD+222)cy/1.2).  Two levers vs the
baseline (128 calls of FD=1024 + 40 ACT drain-copies ~= 157.4us ACT-busy):

1. Fewer, larger exp calls: 96 calls in groups of (3,3,2) k-tiles per
   8-k-tile unit — FD 1536/1536/1024 from two 3-bank PSUM score tiles
   (spA/spB) that double-buffer the score-matmul -> exp pipeline.
2. Attention starts after only kv chunks 0-1 + q chunk 0 are projected
   (the custom softmax has no max subtraction, so ctx/denominator are
   additive over k: two k-halves sweep all (qc, head) units,
   flash-attention style, accumulating ctx^T in SBUF).  The remaining
   kv/q chunks drip into attention's PE/DVE slack.

Per unit (k-half, qc, head): score groups -> exp -> MM2 accumulates
ctx^T[65,512] over the unit's 8 k-tiles into the 1-bank cp region (the V
ones-column yields row sums for free); MM2 groups are pipelined ACROSS
units (global pend, lag 1) so a waiting MM2 never head-of-line blocks the
next unit's scores in the PE queue; one DVE drain adds cp into the SBUF
accumulator.  After the second sweep, normalize: PE-transpose [65,128]
blocks into the util bank, DVE reciprocal of the sums row, scale into a
token-major staging tile, DMA out.

PSUM: spA(3) + spB(3) + cp(1) + util(1) = 8 banks.  Dripped projection /
transpose work rotates between util and cp's inter-unit idle window —
lending the live score tiles' banks instead serializes against the exp
pipeline (coarse-grained dependency tracking) and loses ~10us.

Measured (same-session A/B vs the 157735ns baseline, R=10 repeat NEFFs,
interleaved batches): -13.2us/iter => ~144.5us/core.
"""

import numpy as np
from contextlib import ExitStack

B = 2
L = 2048
D = 1024
H = 16
DK = 64
P = 128
NCORES = 8
CPB = 4  # cores per batch
HPC = H // CPB  # heads per core = 4
COLS = HPC * DK  # 256 output columns per core
CHUNK = 512  # token chunk (max fp32 moving operand)

_CACHE = {}


def _build(L=L, D=D, COLS=COLS, cfg=None, repeat=1):
    import concourse.tile as tile
    from concourse import bacc, masks, mybir

    f32 = mybir.dt.float32
    f32r = mybir.dt.float32r
    Exp = mybir.ActivationFunctionType.Exp
    add_op = mybir.AluOpType.add
    mult_op = mybir.AluOpType.mult

    HL = COLS // DK  # heads handled locally = 4
    FT = D // P  # feature tiles = 8
    TT = L // P  # k token tiles = 16
    TCH = L // CHUNK  # token chunks = 4
    CT = COLS // P  # column tiles = 2
    IT = CHUNK // P  # token tiles per chunk = 4
    VW = DK + 1  # 65: V columns + ones column
    KH = TT // 2  # k-tiles per half (unit) = 8

    cfg = dict(
        dict(
            grp=(3, 3, 2),   # k-tiles per exp call within a unit
            order="kouter",  # kouter | fine
            eager_kv=2,      # kv chunks loaded before attention
            drip_sp=False,   # drip may borrow spA/spB bank 2
            drip_cp=True,    # drip may use cp between units
            q0_early=True,   # emit q0 before kv chunk 1
            warmup=False,    # split qc=0 sweep-0 into half-units
            qa_s1=False,     # defer dripped chunks' q_a to sweep 1
            norm_eng="vector",  # pool | vector for the normalize multiply
            qa_psum_dma=False,  # DMA q_a straight from PSUM
            dvet_kv=(),  # kv chunks loaded via DVE stream-transpose
            dvet_q=(),   # q chunks loaded via DVE stream-transpose
            lag=1,
        ),
        **(cfg or {}),
    )

    nc = bacc.Bacc(
        "TRN2",
        target_bir_lowering=False,
        debug=False,
        num_devices=NCORES,
    )

    x_q = nc.dram_tensor("x_q", [L, D], f32, kind="ExternalInput").ap()
    x_kv = nc.dram_tensor("x_kv", [L, D], f32, kind="ExternalInput").ap()
    w_q = nc.dram_tensor("w_q", [D, COLS], f32, kind="ExternalInput").ap()
    w_k = nc.dram_tensor("w_k", [D, COLS], f32, kind="ExternalInput").ap()
    w_v = nc.dram_tensor("w_v", [D, COLS], f32, kind="ExternalInput").ap()
    b_q = nc.dram_tensor("b_q", [COLS], f32, kind="ExternalInput").ap()
    b_k = nc.dram_tensor("b_k", [COLS], f32, kind="ExternalInput").ap()
    b_v = nc.dram_tensor("b_v", [COLS], f32, kind="ExternalInput").ap()
    q_out = nc.dram_tensor("q_out", [L, COLS], f32, kind="ExternalOutput").ap()
    c_out = nc.dram_tensor("c_out", [L, COLS], f32, kind="ExternalOutput").ap()

    with tile.TileContext(nc) as tc, ExitStack() as ctx:
        singles = ctx.enter_context(tc.tile_pool(name="singles", bufs=1))

        identity = singles.tile([P, P], f32)
        masks.make_identity(nc, identity[:])

        # biases: q/k as per-partition scalars in ^T layout; v broadcast to rows
        bq_sb = singles.tile([P, CT], f32)
        bk_sb = singles.tile([P, CT], f32)
        nc.sync.dma_start(bq_sb[:], b_q.rearrange("(c p) -> p c", p=P))
        nc.sync.dma_start(bk_sb[:], b_k.rearrange("(c p) -> p c", p=P))
        bv_row = singles.tile([1, COLS], f32)
        nc.sync.dma_start(bv_row[:], b_v.rearrange("(o c) -> o c", o=1))
        bv_bcast = singles.tile([P, COLS], f32)
        nc.gpsimd.partition_broadcast(bv_bcast[:], bv_row[:1])

        # weights: [D, COLS] -> [128, FT, COLS], rounded to f32r via DVE
        wq_sb = singles.tile([P, FT * COLS], f32r)
        wk_sb = singles.tile([P, FT * COLS], f32r)
        wv_sb = singles.tile([P, FT * COLS], f32r)
        wq_sb = wq_sb.rearrange("p (f c) -> p f c", f=FT)
        wk_sb = wk_sb.rearrange("p (f c) -> p f c", f=FT)
        wv_sb = wv_sb.rearrange("p (f c) -> p f c", f=FT)
        wstage_pool = ctx.enter_context(tc.tile_pool(name="wstage", bufs=1))
        for wsb, wdr in ((wq_sb, w_q), (wk_sb, w_k), (wv_sb, w_v)):
            wst = wstage_pool.tile([P, FT * COLS], f32, tag="wst")
            wst = wst.rearrange("p (f c) -> p f c", f=FT)
            nc.sync.dma_start(wst[:], wdr.rearrange("(f p) c -> p f c", p=P))
            nc.vector.tensor_copy(wsb[:], wst[:])

        # persistent attention operands
        QT = singles.tile([P, CT * L], f32r)
        KT = singles.tile([P, CT * L], f32r)
        V = singles.tile([P, TT * HL * VW], f32r)
        QT = QT.rearrange("p (c l) -> p c l", c=CT)
        KT = KT.rearrange("p (c l) -> p c l", c=CT)
        V = V.rearrange("p (t h w) -> p t h w", t=TT, h=HL)
        ones1 = singles.tile([P, 1], f32)
        nc.vector.memset(ones1[:], 1.0)
        nc.vector.tensor_copy(
            V[:, :, :, DK : DK + 1], ones1[:].to_broadcast((P, TT, HL, 1))
        )

        # ctx^T accumulator in SBUF: [65, (qc, h, 512)]
        ACC = singles.tile([VW, TCH * HL * CHUNK], f32)
        ACC = ACC.rearrange("p (q h l) -> p q h l", q=TCH, h=HL)

        # ---- PSUM: spA(3) + spB(3) + cp(1) + util(1) = 8 banks ----
        psum = ctx.enter_context(tc.tile_pool(name="psum", bufs=1, space="PSUM"))
        SPN = 3 * CHUNK  # 1536
        spA = psum.tile([P, SPN], f32, tag="spA")
        spB = psum.tile([P, SPN], f32, tag="spB")
        cp = psum.tile([P, CHUNK], f32, tag="cp")
        util = psum.tile([P, CHUNK], f32, tag="util")
        sp = [spA, spB]

        espool = ctx.enter_context(tc.tile_pool(name="es", bufs=2))
        xnat_pool = ctx.enter_context(tc.tile_pool(name="xnat", bufs=4))
        xt_pool = ctx.enter_context(tc.tile_pool(name="xt", bufs=2))
        qnat_pool = ctx.enter_context(tc.tile_pool(name="qnat", bufs=3))
        nrm_pool = ctx.enter_context(tc.tile_pool(name="nrm", bufs=2))
        rec_pool = ctx.enter_context(tc.tile_pool(name="rec", bufs=2))
        ctxb_pool = ctx.enter_context(tc.tile_pool(name="ctxb", bufs=2))

        # ---- PSUM slice provider --------------------------------------
        # Fill phase rotates over all 8 banks; during attention the drip
        # rotates util (+ the spare third banks of spA/spB, which the (3,3,2)
        # group pattern leaves idle most of the time).
        class Provider:
            def __init__(self):
                self.fill = [
                    spA[:, 0:CHUNK], spB[:, 0:CHUNK],
                    spA[:, CHUNK : 2 * CHUNK], spB[:, CHUNK : 2 * CHUNK],
                    spA[:, 2 * CHUNK : 3 * CHUNK], spB[:, 2 * CHUNK : 3 * CHUNK],
                    cp[:, 0:CHUNK], util[:, 0:CHUNK],
                ]
                if cfg["drip_sp"]:
                    self.drip = [
                        util[:, 0:CHUNK],
                        spA[:, 2 * CHUNK : 3 * CHUNK],
                        spB[:, 2 * CHUNK : 3 * CHUNK],
                    ]
                else:
                    self.drip = [util[:, 0:CHUNK]]
                self.dripw = [util[:, 0:CHUNK]]  # warmup: cp busy
                self.fill2 = [util[:, 0:CHUNK], cp[:, 0:CHUNK]]
                self.lst = self.fill
                self.name = "fill"
                self.i = 0
                # cp may be lent to drip ONLY in the window right after a
                # unit's final MM2 + ACC drain (set by mm2_pop) and before
                # the next unit's first MM2 — a cp write anywhere else
                # clobbers the live ctx^T accumulation.
                self.cp_ready = False

            def mode(self, name):
                self.lst = getattr(self, name)
                self.name = name
                self.i = 0

            def get(self):
                if (self.name == "drip" and cfg["drip_cp"]
                        and self.cp_ready):
                    self.cp_ready = False
                    return cp[:, 0:CHUNK]
                s = self.lst[self.i % len(self.lst)]
                self.i += 1
                return s

        prov = Provider()

        # ---- building blocks -------------------------------------------
        def lt_load(xdram, tag, ch):
            tok0 = ch * CHUNK
            xnats = []
            for it in range(IT):
                xn = xnat_pool.tile([P, D], f32, tag="xn", name=f"xn{tag}")
                nc.sync.dma_start(
                    xn[:], xdram[tok0 + it * P : tok0 + (it + 1) * P, :]
                )
                xnats.append(xn)
            xt = xt_pool.tile([P, FT * CHUNK], f32r, tag="xt", name=f"xt{tag}")
            xt = xt.rearrange("p (f l) -> p f l", f=FT)
            return xnats, xt

        def lt_transpose(xnats, xt, ft, drain):
            tp = prov.get()
            for it in range(IT):
                nc.tensor.transpose(
                    tp[:, it * P : (it + 1) * P],
                    xnats[it][:, ft * P : (ft + 1) * P],
                    identity[:],
                )
            drain(xt[:, ft, :], tp[:])

        def load_transpose(xdram, tag, ch, drain):
            xnats, xt = lt_load(xdram, tag, ch)
            for ft in range(FT):
                lt_transpose(xnats, xt, ft, drain)
            return xt

        def load_transpose_dve(xdram, tag, ch):
            """Block-swapped DMA load + DVE StreamTranspose (no PE/PSUM/ACT).

            dst S[32A+v, 32B+u] = X[tok0+32B+v, 128ft+32A+u]; stream-transpose
            of 32x32 blocks then yields X^T exactly.
            """
            tok0 = ch * CHUNK
            xt = xt_pool.tile([P, FT * CHUNK], f32r, tag="xt", name=f"xt{tag}")
            xt = xt.rearrange("p (f l) -> p f l", f=FT)
            for ft in range(FT):
                s = xnat_pool.tile([P, CHUNK], f32, tag="xs", name=f"xs{tag}")
                blk = xdram[tok0 : tok0 + CHUNK, ft * P : (ft + 1) * P]
                swz = blk.rearrange("(b v) (a u) -> a v b u", v=32, u=32)
                dst = s.rearrange("p (b u) -> p b u", u=32)
                for a in range(4):
                    nc.sync.dma_start(dst[a * 32 : (a + 1) * 32], swz[a])
                nc.vector.transpose(xt[:, ft, :], s[:])
            return xt

        def proj_T_ct(wsb, xt, bsb, OUT, ch, ct):
            tok0 = ch * CHUNK
            pp = prov.get()
            for ft in range(FT):
                nc.tensor.matmul(
                    pp[:],
                    wsb[:, ft, ct * P : (ct + 1) * P],
                    xt[:, ft, :],
                    start=(ft == 0),
                    stop=(ft == FT - 1),
                )
            nc.vector.tensor_scalar_add(
                OUT[:, ct, tok0 : tok0 + CHUNK], pp[:], bsb[:, ct : ct + 1]
            )

        def proj_v_it(xt_kv, ch, it):
            pv = prov.get()
            for ft in range(FT):
                nc.tensor.matmul(
                    pv[:, :COLS],
                    xt_kv[:, ft, it * P : (it + 1) * P],
                    wv_sb[:, ft, :],
                    start=(ft == 0),
                    stop=(ft == FT - 1),
                )
            nc.vector.tensor_tensor(
                V[:, ch * IT + it, :, 0:DK],
                pv[:, :COLS].rearrange("p (h w) -> p h w", h=HL),
                bv_bcast[:].rearrange("p (h w) -> p h w", h=HL),
                op=add_op,
            )

        def qa_out_one(ch, ct, it):
            tok0 = ch * CHUNK
            tq = prov.get()
            nc.tensor.transpose(
                tq[:, :P],
                QT[:, ct, tok0 + it * P : tok0 + (it + 1) * P].bitcast(f32),
                identity[:],
            )
            if cfg["qa_psum_dma"]:
                nc.sync.dma_start(
                    q_out[tok0 + it * P : tok0 + (it + 1) * P, ct * P : (ct + 1) * P],
                    tq[:, :P],
                )
            else:
                qn = qnat_pool.tile([P, P], f32, tag="qn", name="qn")
                nc.vector.tensor_copy(qn[:], tq[:, :P])
                nc.sync.dma_start(
                    q_out[tok0 + it * P : tok0 + (it + 1) * P, ct * P : (ct + 1) * P],
                    qn[:],
                )

        # ---- attention unit: (khalf, qc, h), 8 k-tiles, groups (3,3,2) ---
        tilesel = [0]  # alternates spA/spB per score group, globally
        pend = []  # (kts, es, kh, qc, h, last) — MM2 groups pipelined ACROSS units

        def mm2_pop():
            kts, es, kt0, ktn, qc, h, last = pend.pop(0)
            es_off = 0
            for kt in kts:
                nc.tensor.matmul(
                    cp[:VW, :],
                    V[:, kt, h, :],
                    es[:, es_off : es_off + CHUNK],
                    start=(kt == kt0),
                    stop=(kt == kt0 + ktn - 1),
                )
                es_off += CHUNK
            if last:
                # drain ctx^T partial for this unit into the SBUF accumulator
                if kt0 == 0:
                    nc.vector.tensor_copy(ACC[:, qc, h, :], cp[:VW, :])
                else:
                    nc.vector.tensor_tensor(
                        ACC[:, qc, h, :], cp[:VW, :], ACC[:, qc, h, :], op=add_op
                    )
                prov.cp_ready = True
            else:
                prov.cp_ready = False

        def unit(kt0, grp, qc, h, feed=None):
            q0 = qc * CHUNK
            ct, hh = divmod(h, HL // CT)
            rows = slice(hh * DK, (hh + 1) * DK)
            ktn = sum(grp)

            off = 0
            for gi, g in enumerate(grp):
                kts = list(range(kt0 + off, kt0 + off + g))
                t = sp[tilesel[0]]
                tilesel[0] ^= 1
                for j, kt in enumerate(kts):
                    nc.tensor.matmul(
                        t[:, j * CHUNK : (j + 1) * CHUNK],
                        KT[rows, ct, kt * P : (kt + 1) * P],
                        QT[rows, ct, q0 : q0 + CHUNK],
                        start=True,
                        stop=True,
                    )
                es = espool.tile([P, SPN], f32r, tag="es", name="es")
                nc.scalar.activation(
                    es[:, : g * CHUNK], t[:, : g * CHUNK], Exp, scale=0.125
                )
                pend.append((kts, es, kt0, ktn, qc, h, gi == len(grp) - 1))
                if len(pend) > cfg["lag"]:
                    mm2_pop()
                    if feed is not None:
                        feed()
                if feed is not None:
                    feed()
                off += g

        def normalize(qc, h, ctxbuf):
            """ACC[:, qc, h] -> token-major normalized ctx in ctxbuf."""
            tn = prov.get()
            for it in range(IT):
                nc.tensor.transpose(
                    tn[:, it * VW : (it + 1) * VW],
                    ACC[:VW, qc, h, it * P : (it + 1) * P],
                    identity[:VW, :VW],
                )
            nrm = nrm_pool.tile([P, IT * VW], f32, tag="nrm", name="nrm")
            nc.vector.tensor_copy(nrm[:], tn[:, : IT * VW])
            nrmv = nrm[:].rearrange("p (i w) -> p i w", i=IT)
            rec = rec_pool.tile([P, 2 * IT], f32, tag="rec", name="rec")
            recv = rec[:].rearrange("p (x i) -> p x i", x=2)
            nc.vector.tensor_scalar_add(recv[:, 0, :], nrmv[:, :, DK], 1e-8)
            nc.vector.reciprocal(recv[:, 1, :], recv[:, 0, :])
            eng = nc.gpsimd if cfg["norm_eng"] == "pool" else nc.vector
            eng.tensor_tensor(
                ctxbuf[:].rearrange("p (i c) -> p i c", i=IT)[
                    :, :, h * DK : (h + 1) * DK
                ],
                nrmv[:, :, 0:DK],
                recv[:, 1:2, :]
                .rearrange("p x i -> p i x")
                .to_broadcast((P, IT, DK)),
                op=mult_op,
            )

        # ---- drip-unit machinery ----------------------------------------
        # units_q: (gate, closure) FIFO of projection work dripped into
        # attention slack.  units_norm: priority queue of deferred
        # normalize/output work (consumed first — cheap, unblocks SBUF).
        units_q = []
        units_norm = []

        def dvet_ft(xdram, xt, ch, ft, tag):
            tok0 = ch * CHUNK
            s = xnat_pool.tile([P, CHUNK], f32, tag="xs", name=f"xs{tag}")
            blk = xdram[tok0 : tok0 + CHUNK, ft * P : (ft + 1) * P]
            swz = blk.rearrange("(b v) (a u) -> a v b u", v=32, u=32)
            dst = s.rearrange("p (b u) -> p b u", u=32)
            for a in range(4):
                nc.sync.dma_start(dst[a * 32 : (a + 1) * 32], swz[a])
            nc.vector.transpose(xt[:, ft, :], s[:])

        def q_side_units(ch, qa=True):
            g = f"q{ch}"
            if ch in cfg["dvet_q"]:
                xt = xt_pool.tile([P, FT * CHUNK], f32r, tag="xt", name="xtq")
                xt = xt.rearrange("p (f l) -> p f l", f=FT)
                for ft in range(FT):
                    units_q.append(
                        (g, lambda x=xt, f=ft, k=ch: dvet_ft(x_q, x, k, f, "q"))
                    )
            else:
                xnats, xt = lt_load(x_q, "q", ch)
                for ft in range(FT):
                    units_q.append(
                        (g, lambda xn=xnats, x=xt, f=ft: lt_transpose(
                            xn, x, f, nc.vector.tensor_copy
                        ))
                    )
            for ct in range(CT):
                units_q.append(
                    (g, lambda x=xt, c=ct, k=ch: proj_T_ct(wq_sb, x, bq_sb, QT, k, c))
                )
            if qa:
                for ct in range(CT):
                    for it in range(IT):
                        units_q.append(
                            (g, lambda k=ch, c=ct, i=it: qa_out_one(k, c, i))
                        )

        def kv_side_units(ch, gate=None):
            g = gate or f"kv{ch}"
            if ch in cfg["dvet_kv"]:
                xt = xt_pool.tile([P, FT * CHUNK], f32r, tag="xt", name="xtkv")
                xt = xt.rearrange("p (f l) -> p f l", f=FT)
                for ft in range(FT):
                    units_q.append(
                        (g, lambda x=xt, f=ft, k=ch: dvet_ft(x_kv, x, k, f, "kv"))
                    )
            else:
                xnats, xt = lt_load(x_kv, "kv", ch)
                for ft in range(FT):
                    units_q.append(
                        (g, lambda xn=xnats, x=xt, f=ft: lt_transpose(
                            xn, x, f, nc.vector.tensor_copy
                        ))
                    )
            for ct in range(CT):
                units_q.append(
                    (g, lambda x=xt, c=ct, k=ch: proj_T_ct(wk_sb, x, bk_sb, KT, k, c))
                )
            for it in range(IT):
                units_q.append((g, lambda x=xt, k=ch, i=it: proj_v_it(x, k, i)))

        def feed():
            if units_norm:
                units_norm.pop(0)()
            elif units_q:
                units_q.pop(0)[1]()

        def flush(gate):
            """Emit all queued drip work up to and including `gate` —
            required before any unit that reads what the gate produces."""
            while any(g == gate for g, _ in units_q):
                units_q.pop(0)[1]()

        # ---- emission ----------------------------------------------------
        def kv_eager(ch):
            if ch in cfg["dvet_kv"]:
                xt_kv = load_transpose_dve(x_kv, "kv", ch)
            else:
                xt_kv = load_transpose(x_kv, "kv", ch, drain=nc.scalar.copy)
            for ct in range(CT):
                proj_T_ct(wk_sb, xt_kv, bk_sb, KT, ch, ct)
            for it in range(IT):
                proj_v_it(xt_kv, ch, it)

        def emit_kouter():
            EK = cfg["eager_kv"]
            def q0_block():
                if 0 in cfg["dvet_q"]:
                    xt_q0 = load_transpose_dve(x_q, "q", 0)
                else:
                    xt_q0 = load_transpose(x_q, "q", 0, drain=nc.scalar.copy)
                for ct in range(CT):
                    proj_T_ct(wq_sb, xt_q0, bq_sb, QT, 0, ct)
            kv_eager(0)
            q0_block()
            if not cfg["warmup"]:
                prov.mode("fill")
                kv_eager(1)
            prov.mode("dripw")
            for ct in range(CT):
                for it in range(IT):
                    qa_out_one(0, ct, it)
            # drip order matched to first use: q(qc) gates sweep-0 unit
            # 4*qc, kv(2)/kv(3) gate sweep-1; qa for chunks 1-3 runs in
            # sweep 1 (light chain).  Only q1 is queued before the warmup —
            # later queues would exhaust the xn staging slots kv1 needs.
            def queue_rest():
                s1 = cfg["qa_s1"]
                q_side_units(1, qa=not s1)
                q_side_units(2, qa=not s1)
                kv_side_units(2)
                q_side_units(3, qa=not s1)
                kv_side_units(3)
                if s1:
                    for ch in range(1, TCH):
                        for ct in range(CT):
                            for it in range(IT):
                                units_q.append(
                                    (f"qa{ch}",
                                     lambda k=ch, c=ct, i=it: qa_out_one(k, c, i))
                                )
            if cfg["warmup"]:
                # sweep 0 (k-tiles 0..7): qc=0 split into two 4-k-tile
                # half-units — the first needs only kv chunk 0, and its exp
                # stream covers kv chunk 1's load.
                for h in range(HL):
                    unit(0, (3, 1), 0, h, feed=feed)
                while pend:  # cp must be quiescent before lending it out
                    mm2_pop()
                # spA/spB sit idle until round 2 (which needs kv1 anyway) —
                # let kv1 use the full fill rotation
                prov.mode("fill")
                kv_eager(1)
                queue_rest()
                prov.mode("dripw")
                for h in range(HL):
                    unit(4, (3, 1), 0, h, feed=feed)
            else:
                prov.mode("drip")
                queue_rest()
                for h in range(HL):
                    unit(0, cfg["grp"], 0, h, feed=feed)
            prov.mode("drip")
            for qc in range(1, TCH):
                flush(f"q{qc}")
                for h in range(HL):
                    unit(0, cfg["grp"], qc, h, feed=feed)
            for ch in range(EK, TCH):
                flush(f"kv{ch}")
            for qc in range(TCH):
                ctxbuf = ctxb_pool.tile(
                    [P, IT * COLS], f32, tag="ctxb", name="ctxb"
                )
                for h in range(HL):
                    unit(KH, cfg["grp"], qc, h, feed=feed)
                    # defer normalize so it doesn't head-of-line block the
                    # next unit's score matmuls in the PE queue
                    units_norm.append(
                        lambda q=qc, hh=h, cb=ctxbuf: normalize(q, hh, cb)
                    )
                def ctx_dma(q=qc, cb=ctxbuf):
                    cbv = cb[:].rearrange("p (i c) -> p i c", i=IT)
                    for it in range(IT):
                        nc.sync.dma_start(
                            c_out[q * CHUNK + it * P : q * CHUNK + (it + 1) * P, :],
                            cbv[:, it, :],
                        )
                units_norm.append(ctx_dma)
            while pend:
                mm2_pop()
            while units_norm:
                units_norm.pop(0)()
            while units_q:
                units_q.pop(0)[1]()

        for _rep in range(repeat):
            tilesel[0] = 0
            prov.mode("fill")
            emit_kouter()
    nc.compile()
    return nc


def _get_nc():
    if "nc" not in _CACHE:
        _CACHE["nc"] = _build(cfg=_CACHE.get("cfg"))
    return _CACHE["nc"]


def make_in_maps(cdd, his, W_q, b_q, W_k, b_k, W_v, b_v):
    cdd = np.asarray(cdd, dtype=np.float32)
    his = np.asarray(his, dtype=np.float32)
    W_q = np.asarray(W_q, dtype=np.float32)
    W_k = np.asarray(W_k, dtype=np.float32)
    W_v = np.asarray(W_v, dtype=np.float32)
    b_q = np.asarray(b_q, dtype=np.float32)
    b_k = np.asarray(b_k, dtype=np.float32)
    b_v = np.asarray(b_v, dtype=np.float32)
    in_maps = []
    for c in range(NCORES):
        b, hb = divmod(c, CPB)
        sl = slice(hb * COLS, (hb + 1) * COLS)
        in_maps.append(
            {
                "x_q": np.ascontiguousarray(cdd[b]),
                "x_kv": np.ascontiguousarray(his[b]),
                "w_q": np.ascontiguousarray(W_q[:, sl]),
                "w_k": np.ascontiguousarray(W_k[:, sl]),
                "w_v": np.ascontiguousarray(W_v[:, sl]),
                "b_q": np.ascontiguousarray(b_q[sl]),
                "b_k": np.ascontiguousarray(b_k[sl]),
                "b_v": np.ascontiguousarray(b_v[sl]),
            }
        )
    return in_maps


def assemble_outputs(results):
    context = np.zeros((B, L, D), dtype=np.float32)
    q_a = np.zeros((B, L, D), dtype=np.float32)
    for c, out in enumerate(results):
        b, hb = divmod(c, CPB)
        sl = slice(hb * COLS, (hb + 1) * COLS)
        q_a[b, :, sl] = out["q_out"]
        context[b, :, sl] = out["c_out"]
    return (context, q_a)


def kernel(cdd, his, W_q, b_q, W_k, b_k, W_v, b_v):
    from concourse.bass_utils import run_bass_kernel_spmd

    nc = _get_nc()
    in_maps = make_in_maps(cdd, his, W_q, b_q, W_k, b_k, W_v, b_v)

    res = run_bass_kernel_spmd(
        nc, in_maps, core_ids=list(range(NCORES)), trace=_CACHE.get("trace", False)
    )
    _CACHE["last_result"] = res
    return assemble_outputs(res.results)


# revision 31
# speedup vs baseline: 1.6540x; 1.1371x over previous
"""Trainium2 Bass kernel for MultiHeadSelfAttention (cross-attention variant).

Problem: B=2, LQ=LK=2048, D=1024, H=16, d_k=64, fp32.
  q_a = cdd @ W_q + b_q ; k_a = his @ W_k + b_k ; v_a = his @ W_v + b_v
  S = q k^T / 8 ; A = exp(S) / (sum_k exp(S) + 1e-8) ; ctx = A v
  returns (context, q_a)

Sharding (8 cores, no collectives): core c handles batch c//4 and head-block
c%4 (4 heads = 256 columns of W_q/W_k/W_v).  Each core writes disjoint column
slices of both outputs; the host gathers them.

The kernel is ACT(exp)-bound: 16.8M exps/core at 1 elem/cycle/lane @1.2GHz
(~109us floor; each ACTIVATE costs ~(FD+222)cy/1.2).  Two levers vs the
baseline (128 calls of FD=1024 + 40 ACT drain-copies ~= 157.4us ACT-busy):

1. Fewer, larger exp calls: 96 calls in groups of (3,3,2) k-tiles per
   8-k-tile unit — FD 1536/1536/1024 from two 3-bank PSUM score tiles
   (spA/spB) that double-buffer the score-matmul -> exp pipeline.
2. Attention starts after only kv chunks 0-1 + q chunk 0 are projected
   (the custom softmax has no max subtraction, so ctx/denominator are
   additive over k: two k-halves sweep all (qc, head) units,
   flash-attention style, accumulating ctx^T in SBUF).  The remaining
   kv/q chunks drip into attention's PE/DVE slack.

Per unit (k-half, qc, head): score groups -> exp -> MM2 accumulates
ctx^T[65,512] over the unit's 8 k-tiles into the 1-bank cp region (the V
ones-column yields row sums for free); MM2 groups are pipelined ACROSS
units (global pend, lag 1) so a waiting MM2 never head-of-line blocks the
next unit's scores in the PE queue; one DVE drain adds cp into the SBUF
accumulator.  After the second sweep, normalize: PE-transpose [65,128]
blocks into the util bank, DVE reciprocal of the sums row, scale into a
token-major staging tile, DMA out.

PSUM: spA(3) + spB(3) + cp(1) + util(1) = 8 banks.  Dripped projection /
transpose work rotates between util and cp's inter-unit idle window —
lending the live score tiles' banks instead serializes against the exp
pipeline (coarse-grained dependency tracking) and loses ~10us.

The MM2 accumulator alternates between the cp and util banks per unit; the
idle one is that unit's drip target — full-bank safe drip capacity with
chains pipelining across banks on consecutive units.

Measured (same-session A/B vs the 157735ns baseline, R=10 repeat NEFFs,
interleaved batches): about -3us/iter, ~155us/core; matches the cost-model
sim delta (246.9us vs 249.7us).
"""

import numpy as np
from contextlib import ExitStack

B = 2
L = 2048
D = 1024
H = 16
DK = 64
P = 128
NCORES = 8
CPB = 4  # cores per batch
HPC = H // CPB  # heads per core = 4
COLS = HPC * DK  # 256 output columns per core
CHUNK = 512  # token chunk (max fp32 moving operand)

_CACHE = {}


def _build(L=L, D=D, COLS=COLS, cfg=None, repeat=1):
    import concourse.tile as tile
    from concourse import bacc, masks, mybir

    f32 = mybir.dt.float32
    f32r = mybir.dt.float32r
    Exp = mybir.ActivationFunctionType.Exp
    add_op = mybir.AluOpType.add
    mult_op = mybir.AluOpType.mult

    HL = COLS // DK  # heads handled locally = 4
    FT = D // P  # feature tiles = 8
    TT = L // P  # k token tiles = 16
    TCH = L // CHUNK  # token chunks = 4
    CT = COLS // P  # column tiles = 2
    IT = CHUNK // P  # token tiles per chunk = 4
    VW = DK + 1  # 65: V columns + ones column
    KH = TT // 2  # k-tiles per half (unit) = 8

    cfg = dict(
        dict(
            grp=(3, 3, 2),   # k-tiles per exp call within a unit
            order="kouter",  # kouter | fine
            eager_kv=2,      # kv chunks loaded before attention
            drip_sp=False,   # drip may borrow spA/spB bank 2
            drip_cp=True,    # drip may use cp between units
            q0_early=True,   # emit q0 before kv chunk 1
            warmup=False,    # split qc=0 sweep-0 into half-units
            qa_s1=False,     # defer dripped chunks' q_a to sweep 1
            norm_eng="vector",  # pool | vector for the normalize multiply
            qa_psum_dma=False,  # DMA q_a straight from PSUM
            dvet_kv=(),  # kv chunks loaded via DVE stream-transpose
            dvet_q=(),   # q chunks loaded via DVE stream-transpose
            lag=1,
        ),
        **(cfg or {}),
    )

    nc = bacc.Bacc(
        "TRN2",
        target_bir_lowering=False,
        debug=False,
        num_devices=NCORES,
    )

    x_q = nc.dram_tensor("x_q", [L, D], f32, kind="ExternalInput").ap()
    x_kv = nc.dram_tensor("x_kv", [L, D], f32, kind="ExternalInput").ap()
    w_q = nc.dram_tensor("w_q", [D, COLS], f32, kind="ExternalInput").ap()
    w_k = nc.dram_tensor("w_k", [D, COLS], f32, kind="ExternalInput").ap()
    w_v = nc.dram_tensor("w_v", [D, COLS], f32, kind="ExternalInput").ap()
    b_q = nc.dram_tensor("b_q", [COLS], f32, kind="ExternalInput").ap()
    b_k = nc.dram_tensor("b_k", [COLS], f32, kind="ExternalInput").ap()
    b_v = nc.dram_tensor("b_v", [COLS], f32, kind="ExternalInput").ap()
    q_out = nc.dram_tensor("q_out", [L, COLS], f32, kind="ExternalOutput").ap()
    c_out = nc.dram_tensor("c_out", [L, COLS], f32, kind="ExternalOutput").ap()

    with tile.TileContext(nc) as tc, ExitStack() as ctx:
        singles = ctx.enter_context(tc.tile_pool(name="singles", bufs=1))

        identity = singles.tile([P, P], f32)
        masks.make_identity(nc, identity[:])

        # biases: q/k as per-partition scalars in ^T layout; v broadcast to rows
        bq_sb = singles.tile([P, CT], f32)
        bk_sb = singles.tile([P, CT], f32)
        nc.sync.dma_start(bq_sb[:], b_q.rearrange("(c p) -> p c", p=P))
        nc.sync.dma_start(bk_sb[:], b_k.rearrange("(c p) -> p c", p=P))
        bv_row = singles.tile([1, COLS], f32)
        nc.sync.dma_start(bv_row[:], b_v.rearrange("(o c) -> o c", o=1))
        bv_bcast = singles.tile([P, COLS], f32)
        nc.gpsimd.partition_broadcast(bv_bcast[:], bv_row[:1])

        # weights: [D, COLS] -> [128, FT, COLS], rounded to f32r via DVE
        wq_sb = singles.tile([P, FT * COLS], f32r)
        wk_sb = singles.tile([P, FT * COLS], f32r)
        wv_sb = singles.tile([P, FT * COLS], f32r)
        wq_sb = wq_sb.rearrange("p (f c) -> p f c", f=FT)
        wk_sb = wk_sb.rearrange("p (f c) -> p f c", f=FT)
        wv_sb = wv_sb.rearrange("p (f c) -> p f c", f=FT)
        wstage_pool = ctx.enter_context(tc.tile_pool(name="wstage", bufs=1))
        for wsb, wdr in ((wq_sb, w_q), (wk_sb, w_k), (wv_sb, w_v)):
            wst = wstage_pool.tile([P, FT * COLS], f32, tag="wst")
            wst = wst.rearrange("p (f c) -> p f c", f=FT)
            nc.sync.dma_start(wst[:], wdr.rearrange("(f p) c -> p f c", p=P))
            nc.vector.tensor_copy(wsb[:], wst[:])

        # persistent attention operands
        QT = singles.tile([P, CT * L], f32r)
        KT = singles.tile([P, CT * L], f32r)
        V = singles.tile([P, TT * HL * VW], f32r)
        QT = QT.rearrange("p (c l) -> p c l", c=CT)
        KT = KT.rearrange("p (c l) -> p c l", c=CT)
        V = V.rearrange("p (t h w) -> p t h w", t=TT, h=HL)
        ones1 = singles.tile([P, 1], f32)
        nc.vector.memset(ones1[:], 1.0)
        nc.vector.tensor_copy(
            V[:, :, :, DK : DK + 1], ones1[:].to_broadcast((P, TT, HL, 1))
        )

        # ctx^T accumulator in SBUF: [65, (qc, h, 512)]
        ACC = singles.tile([VW, TCH * HL * CHUNK], f32)
        ACC = ACC.rearrange("p (q h l) -> p q h l", q=TCH, h=HL)

        # ---- PSUM: spA(3) + spB(3) + cp(1) + util(1) = 8 banks ----
        psum = ctx.enter_context(tc.tile_pool(name="psum", bufs=1, space="PSUM"))
        SPN = 3 * CHUNK  # 1536
        spA = psum.tile([P, SPN], f32, tag="spA")
        spB = psum.tile([P, SPN], f32, tag="spB")
        cp = psum.tile([P, CHUNK], f32, tag="cp")
        util = psum.tile([P, CHUNK], f32, tag="util")
        sp = [spA, spB]

        espool = ctx.enter_context(tc.tile_pool(name="es", bufs=2))
        xnat_pool = ctx.enter_context(tc.tile_pool(name="xnat", bufs=4))
        xt_pool = ctx.enter_context(tc.tile_pool(name="xt", bufs=2))
        qnat_pool = ctx.enter_context(tc.tile_pool(name="qnat", bufs=3))
        nrm_pool = ctx.enter_context(tc.tile_pool(name="nrm", bufs=2))
        rec_pool = ctx.enter_context(tc.tile_pool(name="rec", bufs=2))
        ctxb_pool = ctx.enter_context(tc.tile_pool(name="ctxb", bufs=2))

        # ---- PSUM slice provider --------------------------------------
        # Fill phase rotates over all 8 banks; during attention the drip
        # rotates util (+ the spare third banks of spA/spB, which the (3,3,2)
        # group pattern leaves idle most of the time).
        class Provider:
            def __init__(self):
                self.fill = [
                    spA[:, 0:CHUNK], spB[:, 0:CHUNK],
                    spA[:, CHUNK : 2 * CHUNK], spB[:, CHUNK : 2 * CHUNK],
                    spA[:, 2 * CHUNK : 3 * CHUNK], spB[:, 2 * CHUNK : 3 * CHUNK],
                    cp[:, 0:CHUNK], util[:, 0:CHUNK],
                ]
                if cfg["drip_sp"]:
                    self.drip = [
                        util[:, 0:CHUNK],
                        spA[:, 2 * CHUNK : 3 * CHUNK],
                        spB[:, 2 * CHUNK : 3 * CHUNK],
                    ]
                else:
                    self.drip = [util[:, 0:CHUNK]]
                self.dripw = [util[:, 0:CHUNK]]  # warmup: cp busy
                self.fill2 = [util[:, 0:CHUNK], cp[:, 0:CHUNK]]
                self.lst = self.fill
                self.name = "fill"
                self.i = 0

            def mode(self, name):
                self.lst = getattr(self, name)
                self.name = name
                self.i = 0

            def get(self):
                if self.name in ("drip", "dripw"):
                    # set per-unit: the accumulator bank NOT in use this unit
                    return self.drip[0]
                s = self.lst[self.i % len(self.lst)]
                self.i += 1
                return s

        prov = Provider()

        # ---- building blocks -------------------------------------------
        def lt_load(xdram, tag, ch):
            tok0 = ch * CHUNK
            xnats = []
            for it in range(IT):
                xn = xnat_pool.tile([P, D], f32, tag="xn", name=f"xn{tag}")
                nc.sync.dma_start(
                    xn[:], xdram[tok0 + it * P : tok0 + (it + 1) * P, :]
                )
                xnats.append(xn)
            xt = xt_pool.tile([P, FT * CHUNK], f32r, tag="xt", name=f"xt{tag}")
            xt = xt.rearrange("p (f l) -> p f l", f=FT)
            return xnats, xt

        def lt_transpose(xnats, xt, ft, drain):
            tp = prov.get()
            for it in range(IT):
                nc.tensor.transpose(
                    tp[:, it * P : (it + 1) * P],
                    xnats[it][:, ft * P : (ft + 1) * P],
                    identity[:],
                )
            drain(xt[:, ft, :], tp[:])

        def load_transpose(xdram, tag, ch, drain):
            xnats, xt = lt_load(xdram, tag, ch)
            for ft in range(FT):
                lt_transpose(xnats, xt, ft, drain)
            return xt

        def load_transpose_dve(xdram, tag, ch):
            """Block-swapped DMA load + DVE StreamTranspose (no PE/PSUM/ACT).

            dst S[32A+v, 32B+u] = X[tok0+32B+v, 128ft+32A+u]; stream-transpose
            of 32x32 blocks then yields X^T exactly.
            """
            tok0 = ch * CHUNK
            xt = xt_pool.tile([P, FT * CHUNK], f32r, tag="xt", name=f"xt{tag}")
            xt = xt.rearrange("p (f l) -> p f l", f=FT)
            for ft in range(FT):
                s = xnat_pool.tile([P, CHUNK], f32, tag="xs", name=f"xs{tag}")
                blk = xdram[tok0 : tok0 + CHUNK, ft * P : (ft + 1) * P]
                swz = blk.rearrange("(b v) (a u) -> a v b u", v=32, u=32)
                dst = s.rearrange("p (b u) -> p b u", u=32)
                for a in range(4):
                    nc.sync.dma_start(dst[a * 32 : (a + 1) * 32], swz[a])
                nc.vector.transpose(xt[:, ft, :], s[:])
            return xt

        def proj_T_ct(wsb, xt, bsb, OUT, ch, ct):
            tok0 = ch * CHUNK
            pp = prov.get()
            for ft in range(FT):
                nc.tensor.matmul(
                    pp[:],
                    wsb[:, ft, ct * P : (ct + 1) * P],
                    xt[:, ft, :],
                    start=(ft == 0),
                    stop=(ft == FT - 1),
                )
            nc.vector.tensor_scalar_add(
                OUT[:, ct, tok0 : tok0 + CHUNK], pp[:], bsb[:, ct : ct + 1]
            )

        def proj_v_it(xt_kv, ch, it):
            pv = prov.get()
            for ft in range(FT):
                nc.tensor.matmul(
                    pv[:, :COLS],
                    xt_kv[:, ft, it * P : (it + 1) * P],
                    wv_sb[:, ft, :],
                    start=(ft == 0),
                    stop=(ft == FT - 1),
                )
            nc.vector.tensor_tensor(
                V[:, ch * IT + it, :, 0:DK],
                pv[:, :COLS].rearrange("p (h w) -> p h w", h=HL),
                bv_bcast[:].rearrange("p (h w) -> p h w", h=HL),
                op=add_op,
            )

        def qa_out_one(ch, ct, it):
            tok0 = ch * CHUNK
            tq = prov.get()
            nc.tensor.transpose(
                tq[:, :P],
                QT[:, ct, tok0 + it * P : tok0 + (it + 1) * P].bitcast(f32),
                identity[:],
            )
            if cfg["qa_psum_dma"]:
                nc.sync.dma_start(
                    q_out[tok0 + it * P : tok0 + (it + 1) * P, ct * P : (ct + 1) * P],
                    tq[:, :P],
                )
            else:
                qn = qnat_pool.tile([P, P], f32, tag="qn", name="qn")
                nc.vector.tensor_copy(qn[:], tq[:, :P])
                nc.sync.dma_start(
                    q_out[tok0 + it * P : tok0 + (it + 1) * P, ct * P : (ct + 1) * P],
                    qn[:],
                )

        # ---- attention unit: (khalf, qc, h), 8 k-tiles, groups (3,3,2) ---
        tilesel = [0]  # alternates spA/spB per score group, globally
        pend = []  # (kts, es, kh, qc, h, last) — MM2 groups pipelined ACROSS units

        def mm2_pop():
            kts, es, kt0, ktn, qc, h, last, acc = pend.pop(0)
            es_off = 0
            for kt in kts:
                nc.tensor.matmul(
                    acc[:VW, :],
                    V[:, kt, h, :],
                    es[:, es_off : es_off + CHUNK],
                    start=(kt == kt0),
                    stop=(kt == kt0 + ktn - 1),
                )
                es_off += CHUNK
            if last:
                # drain ctx^T partial for this unit into the SBUF accumulator
                if kt0 == 0:
                    nc.vector.tensor_copy(ACC[:, qc, h, :], acc[:VW, :])
                else:
                    nc.vector.tensor_tensor(
                        ACC[:, qc, h, :], acc[:VW, :], ACC[:, qc, h, :], op=add_op
                    )

        acc_sel = [0]  # alternates cp/util as the MM2 accumulator per unit

        def unit(kt0, grp, qc, h, feed=None):
            q0 = qc * CHUNK
            ct, hh = divmod(h, HL // CT)
            rows = slice(hh * DK, (hh + 1) * DK)
            ktn = sum(grp)
            acc = [cp, util][acc_sel[0]]
            # the other bank is this unit's drip target
            prov.drip = [[util, cp][acc_sel[0]][:, 0:CHUNK]]
            acc_sel[0] ^= 1

            off = 0
            for gi, g in enumerate(grp):
                kts = list(range(kt0 + off, kt0 + off + g))
                t = sp[tilesel[0]]
                tilesel[0] ^= 1
                for j, kt in enumerate(kts):
                    nc.tensor.matmul(
                        t[:, j * CHUNK : (j + 1) * CHUNK],
                        KT[rows, ct, kt * P : (kt + 1) * P],
                        QT[rows, ct, q0 : q0 + CHUNK],
                        start=True,
                        stop=True,
                    )
                es = espool.tile([P, SPN], f32r, tag="es", name="es")
                nc.scalar.activation(
                    es[:, : g * CHUNK], t[:, : g * CHUNK], Exp, scale=0.125
                )
                pend.append((kts, es, kt0, ktn, qc, h, gi == len(grp) - 1, acc))
                if len(pend) > cfg["lag"]:
                    mm2_pop()
                    if feed is not None:
                        feed()
                if feed is not None:
                    feed()
                off += g

        def normalize(qc, h, ctxbuf):
            """ACC[:, qc, h] -> token-major normalized ctx in ctxbuf."""
            tn = prov.get()
            for it in range(IT):
                nc.tensor.transpose(
                    tn[:, it * VW : (it + 1) * VW],
                    ACC[:VW, qc, h, it * P : (it + 1) * P],
                    identity[:VW, :VW],
                )
            nrm = nrm_pool.tile([P, IT * VW], f32, tag="nrm", name="nrm")
            nc.vector.tensor_copy(nrm[:], tn[:, : IT * VW])
            nrmv = nrm[:].rearrange("p (i w) -> p i w", i=IT)
            rec = rec_pool.tile([P, 2 * IT], f32, tag="rec", name="rec")
            recv = rec[:].rearrange("p (x i) -> p x i", x=2)
            nc.vector.tensor_scalar_add(recv[:, 0, :], nrmv[:, :, DK], 1e-8)
            nc.vector.reciprocal(recv[:, 1, :], recv[:, 0, :])
            eng = nc.gpsimd if cfg["norm_eng"] == "pool" else nc.vector
            eng.tensor_tensor(
                ctxbuf[:].rearrange("p (i c) -> p i c", i=IT)[
                    :, :, h * DK : (h + 1) * DK
                ],
                nrmv[:, :, 0:DK],
                recv[:, 1:2, :]
                .rearrange("p x i -> p i x")
                .to_broadcast((P, IT, DK)),
                op=mult_op,
            )

        # ---- drip-unit machinery ----------------------------------------
        # units_q: (gate, closure) FIFO of projection work dripped into
        # attention slack.  units_norm: priority queue of deferred
        # normalize/output work (consumed first — cheap, unblocks SBUF).
        units_q = []
        units_norm = []

        def dvet_ft(xdram, xt, ch, ft, tag):
            tok0 = ch * CHUNK
            s = xnat_pool.tile([P, CHUNK], f32, tag="xs", name=f"xs{tag}")
            blk = xdram[tok0 : tok0 + CHUNK, ft * P : (ft + 1) * P]
            swz = blk.rearrange("(b v) (a u) -> a v b u", v=32, u=32)
            dst = s.rearrange("p (b u) -> p b u", u=32)
            for a in range(4):
                nc.sync.dma_start(dst[a * 32 : (a + 1) * 32], swz[a])
            nc.vector.transpose(xt[:, ft, :], s[:])

        def q_side_units(ch, qa=True):
            g = f"q{ch}"
            if ch in cfg["dvet_q"]:
                xt = xt_pool.tile([P, FT * CHUNK], f32r, tag="xt", name="xtq")
                xt = xt.rearrange("p (f l) -> p f l", f=FT)
                for ft in range(FT):
                    units_q.append(
                        (g, lambda x=xt, f=ft, k=ch: dvet_ft(x_q, x, k, f, "q"))
                    )
            else:
                xnats, xt = lt_load(x_q, "q", ch)
                for ft in range(FT):
                    units_q.append(
                        (g, lambda xn=xnats, x=xt, f=ft: lt_transpose(
                            xn, x, f, nc.vector.tensor_copy
                        ))
                    )
            for ct in range(CT):
                units_q.append(
                    (g, lambda x=xt, c=ct, k=ch: proj_T_ct(wq_sb, x, bq_sb, QT, k, c))
                )
            if qa:
                for ct in range(CT):
                    for it in range(IT):
                        units_q.append(
                            (g, lambda k=ch, c=ct, i=it: qa_out_one(k, c, i))
                        )

        def kv_side_units(ch, gate=None):
            g = gate or f"kv{ch}"
            if ch in cfg["dvet_kv"]:
                xt = xt_pool.tile([P, FT * CHUNK], f32r, tag="xt", name="xtkv")
                xt = xt.rearrange("p (f l) -> p f l", f=FT)
                for ft in range(FT):
                    units_q.append(
                        (g, lambda x=xt, f=ft, k=ch: dvet_ft(x_kv, x, k, f, "kv"))
                    )
            else:
                xnats, xt = lt_load(x_kv, "kv", ch)
                for ft in range(FT):
                    units_q.append(
                        (g, lambda xn=xnats, x=xt, f=ft: lt_transpose(
                            xn, x, f, nc.vector.tensor_copy
                        ))
                    )
            for ct in range(CT):
                units_q.append(
                    (g, lambda x=xt, c=ct, k=ch: proj_T_ct(wk_sb, x, bk_sb, KT, k, c))
                )
            for it in range(IT):
                units_q.append((g, lambda x=xt, k=ch, i=it: proj_v_it(x, k, i)))

        def feed():
            if units_norm:
                units_norm.pop(0)()
            elif units_q:
                units_q.pop(0)[1]()

        def flush(gate):
            """Emit all queued drip work up to and including `gate` —
            required before any unit that reads what the gate produces."""
            while any(g == gate for g, _ in units_q):
                units_q.pop(0)[1]()

        # ---- emission ----------------------------------------------------
        def kv_eager(ch):
            if ch in cfg["dvet_kv"]:
                xt_kv = load_transpose_dve(x_kv, "kv", ch)
            else:
                xt_kv = load_transpose(x_kv, "kv", ch, drain=nc.scalar.copy)
            for ct in range(CT):
                proj_T_ct(wk_sb, xt_kv, bk_sb, KT, ch, ct)
            for it in range(IT):
                proj_v_it(xt_kv, ch, it)

        def emit_kouter():
            EK = cfg["eager_kv"]
            def q0_block():
                if 0 in cfg["dvet_q"]:
                    xt_q0 = load_transpose_dve(x_q, "q", 0)
                else:
                    xt_q0 = load_transpose(x_q, "q", 0, drain=nc.scalar.copy)
                for ct in range(CT):
                    proj_T_ct(wq_sb, xt_q0, bq_sb, QT, 0, ct)
            kv_eager(0)
            q0_block()
            if not cfg["warmup"]:
                prov.mode("fill")
                kv_eager(1)
            prov.mode("dripw")
            for ct in range(CT):
                for it in range(IT):
                    qa_out_one(0, ct, it)
            # drip order matched to first use: q(qc) gates sweep-0 unit
            # 4*qc, kv(2)/kv(3) gate sweep-1; qa for chunks 1-3 runs in
            # sweep 1 (light chain).  Only q1 is queued before the warmup —
            # later queues would exhaust the xn staging slots kv1 needs.
            def queue_rest():
                s1 = cfg["qa_s1"]
                q_side_units(1, qa=not s1)
                q_side_units(2, qa=not s1)
                kv_side_units(2)
                q_side_units(3, qa=not s1)
                kv_side_units(3)
                if s1:
                    for ch in range(1, TCH):
                        for ct in range(CT):
                            for it in range(IT):
                                units_q.append(
                                    (f"qa{ch}",
                                     lambda k=ch, c=ct, i=it: qa_out_one(k, c, i))
                                )
            if cfg["warmup"]:
                # sweep 0 (k-tiles 0..7): qc=0 split into two 4-k-tile
                # half-units — the first needs only kv chunk 0, and its exp
                # stream covers kv chunk 1's load.
                for h in range(HL):
                    unit(0, (3, 1), 0, h, feed=feed)
                while pend:  # cp must be quiescent before lending it out
                    mm2_pop()
                # spA/spB sit idle until round 2 (which needs kv1 anyway) —
                # let kv1 use the full fill rotation
                prov.mode("fill")
                kv_eager(1)
                queue_rest()
                prov.mode("dripw")
                for h in range(HL):
                    unit(4, (3, 1), 0, h, feed=feed)
            else:
                prov.mode("drip")
                queue_rest()
                for h in range(HL):
                    unit(0, cfg["grp"], 0, h, feed=feed)
            prov.mode("drip")
            for qc in range(1, TCH):
                flush(f"q{qc}")
                for h in range(HL):
                    unit(0, cfg["grp"], qc, h, feed=feed)
            for ch in range(EK, TCH):
                flush(f"kv{ch}")
            for qc in range(TCH):
                ctxbuf = ctxb_pool.tile(
                    [P, IT * COLS], f32, tag="ctxb", name="ctxb"
                )
                for h in range(HL):
                    unit(KH, cfg["grp"], qc, h, feed=feed)
                    # defer normalize so it doesn't head-of-line block the
                    # next unit's score matmuls in the PE queue
                    units_norm.append(
                        lambda q=qc, hh=h, cb=ctxbuf: normalize(q, hh, cb)
                    )
                def ctx_dma(q=qc, cb=ctxbuf):
                    cbv = cb[:].rearrange("p (i c) -> p i c", i=IT)
                    for it in range(IT):
                        nc.sync.dma_start(
                            c_out[q * CHUNK + it * P : q * CHUNK + (it + 1) * P, :],
                            cbv[:, it, :],
                        )
                units_norm.append(ctx_dma)
            while pend:
                mm2_pop()
            while units_norm:
                units_norm.pop(0)()
            while units_q:
                units_q.pop(0)[1]()

        for _rep in range(repeat):
            tilesel[0] = 0
            prov.mode("fill")
            emit_kouter()
    nc.compile()
    return nc


def _get_nc():
    if "nc" not in _CACHE:
        _CACHE["nc"] = _build(cfg=_CACHE.get("cfg"))
    return _CACHE["nc"]


def make_in_maps(cdd, his, W_q, b_q, W_k, b_k, W_v, b_v):
    cdd = np.asarray(cdd, dtype=np.float32)
    his = np.asarray(his, dtype=np.float32)
    W_q = np.asarray(W_q, dtype=np.float32)
    W_k = np.asarray(W_k, dtype=np.float32)
    W_v = np.asarray(W_v, dtype=np.float32)
    b_q = np.asarray(b_q, dtype=np.float32)
    b_k = np.asarray(b_k, dtype=np.float32)
    b_v = np.asarray(b_v, dtype=np.float32)
    in_maps = []
    for c in range(NCORES):
        b, hb = divmod(c, CPB)
        sl = slice(hb * COLS, (hb + 1) * COLS)
        in_maps.append(
            {
                "x_q": np.ascontiguousarray(cdd[b]),
                "x_kv": np.ascontiguousarray(his[b]),
                "w_q": np.ascontiguousarray(W_q[:, sl]),
                "w_k": np.ascontiguousarray(W_k[:, sl]),
                "w_v": np.ascontiguousarray(W_v[:, sl]),
                "b_q": np.ascontiguousarray(b_q[sl]),
                "b_k": np.ascontiguousarray(b_k[sl]),
                "b_v": np.ascontiguousarray(b_v[sl]),
            }
        )
    return in_maps


def assemble_outputs(results):
    context = np.zeros((B, L, D), dtype=np.float32)
    q_a = np.zeros((B, L, D), dtype=np.float32)
    for c, out in enumerate(results):
        b, hb = divmod(c, CPB)
        sl = slice(hb * COLS, (hb + 1) * COLS)
        q_a[b, :, sl] = out["q_out"]
        context[b, :, sl] = out["c_out"]
    return (context, q_a)


def kernel(cdd, his, W_q, b_q, W_k, b_k, W_v, b_v):
    from concourse.bass_utils import run_bass_kernel_spmd

    nc = _get_nc()
    in_maps = make_in_maps(cdd, his, W_q, b_q, W_k, b_k, W_v, b_v)

    res = run_bass_kernel_spmd(
        nc, in_maps, core_ids=list(range(NCORES)), trace=_CACHE.get("trace", False)
    )
    _CACHE["last_result"] = res
    return assemble_outputs(res.results)
